# revision 15
# baseline (speedup 1.0000x reference)
"""Nadaraya-Watson kernel regression (retrieval_knn) on 8 NeuronCores.

out[b,d] = sum_n y[n,d] G((Xw[n,d]-Zw[b,d])/h) / sum_n G(...),
G(z) = exp(-z^2/2); Zw = mlp(x), Xw = mlp(calc_X).

Algorithm (Taylor-moment factorization):
  With u = Xw/h [N,16], v = Zw/h [B,16]:
    G = e^{-u^2/2} e^{uv} e^{-v^2/2}; the e^{-v^2/2} cancels in num/den.
  Expand e^{uv} = sum_k (uv)^k / k! (|uv| <= ~4.1 here, K=12 -> ~5e-6):
    num[b,d] = sum_k v[b,d]^k A[k,d],  A[k,d] = sum_n y phi u^k / k!
    den[b,d] = sum_k v[b,d]^k B[k,d],  B[k,d] = sum_n   phi u^k / k!
  phi = e^{-u^2/2}.  The O(B N D) elementwise pass becomes O(N D K)
  moment accumulation + O(B D K) Horner evaluation.

Sharding: queries data-parallel (64/core).  Moments either
  - VARIANT="shard": each core does N/8 of the moment sum + 1.6KB AllReduce
  - VARIANT="repl":  each core computes full-N moments, no collective.

Per-core layout: moment work in [128, NSL/8] "lay" space with partition
p = j*16+d (j = n-block, d = output dim); per-op accum_out row-sums give
unnormalized moments per (j,d); a 0/1-selector matmul contracts j.
Horner in [128, 8]: partition p = r*16+d, free g; query b = 8g+r.
"""
import sys
sys.path.insert(0, '/opt/trn_rl_repo')
import numpy as np
from concourse import bass, tile, bacc, mybir
from concourse.bass_utils import run_bass_kernel_spmd

F32 = mybir.dt.float32
F32R = mybir.dt.float32r
AF = mybir.ActivationFunctionType
ALU = mybir.AluOpType

B, N, DIN, DMID, DOUT = 512, 8192, 128, 256, 16
NCORES = 8
BC = B // NCORES            # 64 queries per core
NG = BC // 8                # 8 query groups of 8
K = 12                      # Taylor order; NK moments per output dim
NK = K + 1

VARIANT = "shard"           # "shard" (N/8 + AllReduce) | "repl" (full N)


def build_kernel(reps=1, variant=None):
    variant = variant or VARIANT
    nsl = N // NCORES if variant == "shard" else N
    jw = nsl // 8           # lay-space width; partition p = j*16+d
    nch = 1024 if variant == "shard" else 512   # MLP chunk (psum-limited)
    nchunk = nsl // nch

    nc = bacc.Bacc(None, target_bir_lowering=False)

    xT_d = nc.dram_tensor("xT", [DIN, BC], F32, kind="ExternalInput")
    XT_d = nc.dram_tensor("XTs", [DIN, nsl], F32R, kind="ExternalInput")
    Y_d = nc.dram_tensor("Y_lay", [128, jw], F32, kind="ExternalInput")
    W1T_d = nc.dram_tensor("W1T", [DIN, DMID], F32R, kind="ExternalInput")
    W2Ta_d = nc.dram_tensor("W2Ta", [DIN, DOUT], F32R, kind="ExternalInput")
    W2Tb_d = nc.dram_tensor("W2Tb", [DIN, DOUT], F32R, kind="ExternalInput")
    sel_d = nc.dram_tensor("sel16", [128, DOUT], F32, kind="ExternalInput")
    invf_d = nc.dram_tensor("invf", [2 * NK], F32, kind="ExternalInput")
    out_d = nc.dram_tensor("y_out", [BC, DOUT], F32, kind="ExternalOutput")

    def c32(ap):
        return ap.bitcast(F32)

    with tile.TileContext(nc) as tc:
      for _rep in range(reps):
        with (
            tc.tile_pool(name="dram", bufs=1, space="DRAM") as dram,
            tc.tile_pool(name="const", bufs=1) as cpool,
            tc.tile_pool(name="lay", bufs=1) as lay,
            tc.tile_pool(name="mlp", bufs=2) as mlp,
            tc.tile_pool(name="chain", bufs=2) as chp,
            tc.tile_pool(name="php", bufs=(2 if variant == "shard" else 4),
                         space="PSUM") as php,
            tc.tile_pool(name="pup", bufs=(1 if variant == "shard" else 2),
                         space="PSUM") as pup,
            tc.tile_pool(name="psm", bufs=2, space="PSUM") as psm,
        ):
            # ---------- constant loads ----------
            W1T = cpool.tile([DIN, DMID], F32R)
            nc.sync.dma_start(W1T[:], W1T_d[:])
            W2Ta = cpool.tile([DIN, DOUT], F32R)
            nc.sync.dma_start(W2Ta[:], W2Ta_d[:])
            W2Tb = cpool.tile([DIN, DOUT], F32R)
            nc.sync.dma_start(W2Tb[:], W2Tb_d[:])
            xT = cpool.tile([DIN, BC], F32)
            nc.sync.dma_start(xT[:], xT_d[:])
            sel16 = cpool.tile([128, DOUT], F32)
            nc.sync.dma_start(sel16[:], sel_d[:])
            Y_lay = lay.tile([128, jw], F32, tag="Y")
            nc.sync.dma_start(Y_lay[:], Y_d[:])

            # ---------- query MLP: vT [16, 64] = Zw.T / h ----------
            pq = psm.tile([128, 128], F32, tag="sm")
            for j in range(2):
                nc.tensor.matmul(pq[:, 64 * j:64 * j + 64],
                                 c32(W1T[:, 128 * j:128 * j + 128]), xT[:])
            HqT = cpool.tile([128, 128], F32)
            nc.scalar.activation(HqT[:], pq[:], AF.Relu)
            pz = psm.tile([128, 128], F32, tag="sm")
            nc.tensor.matmul(pz[0:DOUT, 0:BC], c32(W2Ta[:]), HqT[:, 0:64],
                             start=True, stop=False)
            nc.tensor.matmul(pz[0:DOUT, 0:BC], c32(W2Tb[:]), HqT[:, 64:128],
                             start=False, stop=True)
            vT = cpool.tile([DOUT, BC], F32)
            nc.scalar.activation(vT[:], pz[0:DOUT, 0:BC], AF.Copy)
            # v_lay[16r+d, g] = vT[d, 8g+r]
            vcol_dram = dram.tile([128, NG], F32)
            nc.sync.dma_start(
                bass.AP(vcol_dram[:].tensor, 0,
                        [[8, DOUT], [1, NG], [16 * NG, 8]]),
                vT[:].rearrange("d (g r) -> d g r", g=NG))
            v_lay = cpool.tile([128, NG], F32)
            nc.sync.dma_start(v_lay[:], vcol_dram[:])

            # ---------- calc MLP -> u_dram in lay order ----------
            u_dram = dram.tile([128, jw], F32)
            for q in range(nchunk):
                XTs = mlp.tile([DIN, nch], F32R, tag="xt")
                nc.sync.dma_start(
                    XTs[:], bass.AP(XT_d[:].tensor, q * nch,
                                    [[nsl, DIN], [1, nch]]))
                HT = mlp.tile([128, 2, nch], F32R, tag="ht")
                for j in range(2):
                    ph = php.tile([128, nch], F32, tag="ph")
                    for s in range(nch // 512):
                        nc.tensor.matmul(
                            ph[:, 512 * s:512 * (s + 1)],
                            W1T[:, 128 * j:128 * j + 128],
                            XTs[:, 512 * s:512 * (s + 1)])
                    if j == 0:
                        nc.scalar.activation(HT[:, 0, :], ph[:], AF.Relu)
                    else:
                        nc.vector.tensor_scalar_max(HT[:, 1, :], ph[:], 0.0)
                pu = pup.tile([128, nch], F32, tag="pu")
                for s in range(nch // 512):
                    sl = slice(512 * s, 512 * (s + 1))
                    nc.tensor.matmul(pu[0:DOUT, sl], W2Ta[:],
                                     HT[:, 0, sl], start=True, stop=False)
                    nc.tensor.matmul(pu[0:DOUT, sl], W2Tb[:],
                                     HT[:, 1, sl], start=False, stop=True)
                u16 = mlp.tile([DOUT, nch], F32, tag="u16")
                nc.scalar.activation(u16[:], pu[0:DOUT, :], AF.Copy)
                # scatter chunk to u_dram: flat[(j*16+d)*jw + m] = u[d, n]
                if nch <= jw:
                    j0, moff = (q * nch) // jw, (q * nch) % jw
                    nc.sync.dma_start(
                        bass.AP(u_dram[:].tensor, j0 * 16 * jw + moff,
                                [[jw, DOUT], [1, nch]]),
                        u16[:])
                else:
                    nc.sync.dma_start(
                        bass.AP(u_dram[:].tensor, 0,
                                [[jw, DOUT], [16 * jw, nch // jw], [1, jw]]),
                        u16[:].rearrange("d (j m) -> d j m", m=jw))
            u_lay = lay.tile([128, jw], F32, tag="u")
            nc.sync.dma_start(u_lay[:], u_dram[:])

            # ---------- moments: Mraw[k,d] = sum_n u^k phi (1/k! later) ----
            # Mpart cols 0..NK-1 = B (phi-weighted), NK..2NK-1 = A (y phi)
            Mpart = lay.tile([128, 2 * NK], F32, tag="M")
            usq = chp.tile([128, jw], F32, tag="c0")
            nc.vector.scalar_tensor_tensor(usq[:], u_lay[:], -0.5, u_lay[:],
                                           op0=ALU.mult, op1=ALU.mult)
            phi = lay.tile([128, jw], F32, tag="phi")
            nc.scalar.activation(phi[:], usq[:], AF.Exp,
                                 accum_out=Mpart[:, 0:1])
            u2 = lay.tile([128, jw], F32, tag="u2")
            nc.gpsimd.tensor_mul(u2[:], u_lay[:], u_lay[:])
            yphi = lay.tile([128, jw], F32, tag="yphi")
            nc.vector.scalar_tensor_tensor(yphi[:], phi[:], 1.0, Y_lay[:],
                                           op0=ALU.bypass, op1=ALU.mult,
                                           accum_out=Mpart[:, NK:NK + 1])

            def dve_chain(src, step, base, col0):
                """DVE: t = prev*step, accum -> Mpart col; k = base,base+2,.."""
                prev = src
                for k in range(base + 2, NK, 2):
                    t = chp.tile([128, jw], F32, tag=f"c{col0}{base}")
                    nc.vector.scalar_tensor_tensor(
                        t[:], prev[:], 1.0, step[:],
                        op0=ALU.bypass, op1=ALU.mult,
                        accum_out=Mpart[:, col0 + k:col0 + k + 1])
                    prev = t

            # A (numerator) chains fully on DVE with fused row-sums
            Ao1 = chp.tile([128, jw], F32, tag="ao")
            nc.vector.scalar_tensor_tensor(Ao1[:], yphi[:], 1.0, u_lay[:],
                                           op0=ALU.bypass, op1=ALU.mult,
                                           accum_out=Mpart[:, NK + 1:NK + 2])
            dve_chain(yphi, u2, 0, NK)     # A even
            dve_chain(Ao1, u2, 1, NK)      # A odd
            # B even on DVE; B odd products on Pool + ACT copy row-sums
            dve_chain(phi, u2, 0, 0)       # B even
            pprev = chp.tile([128, jw], F32, tag="bo")
            nc.gpsimd.tensor_mul(pprev[:], phi[:], u_lay[:])
            trash = chp.tile([128, jw], F32, tag="tr")
            nc.scalar.activation(trash[:], pprev[:], AF.Copy,
                                 accum_out=Mpart[:, 1:2])
            for k in range(3, NK, 2):
                t = chp.tile([128, jw], F32, tag="bo")
                nc.gpsimd.tensor_mul(t[:], pprev[:], u2[:])
                trash = chp.tile([128, jw], F32, tag="tr")
                nc.scalar.activation(trash[:], t[:], AF.Copy,
                                     accum_out=Mpart[:, k:k + 1])
                pprev = t

            # j-reduction: Mred[d, col] = sum_j Mpart[j*16+d, col]
            pm = psm.tile([128, 128], F32, tag="sm")
            nc.tensor.matmul(pm[0:DOUT, 0:2 * NK], sel16[:], Mpart[:])
            Msb = cpool.tile([DOUT, 2 * NK], F32)
            nc.scalar.activation(Msb[:], pm[0:DOUT, 0:2 * NK], AF.Copy)
            m_loc = dram.tile([DOUT, 2 * NK], F32)
            nc.sync.dma_start(m_loc[:], Msb[:])
            if variant == "shard":
                m_red = dram.tile([DOUT, 2 * NK], F32)
                nc.gpsimd.collective_compute(
                    "AllReduce", ALU.add,
                    replica_groups=[list(range(NCORES))],
                    ins=[m_loc[:].opt()],
                    outs=[m_red[:].opt()])
            else:
                m_red = m_loc
            # C[p=(r,d), col] = Mred[d, col]/k!, replicated over r
            Craw = cpool.tile([128, 2 * NK], F32)
            nc.sync.dma_start(
                Craw[:], bass.AP(m_red[:].tensor, 0,
                                 [[0, 8], [2 * NK, DOUT], [1, 2 * NK]]))
            invf = cpool.tile([128, 2 * NK], F32)
            nc.sync.dma_start(
                invf[:], bass.AP(invf_d[:].tensor, 0, [[0, 128], [1, 2 * NK]]))
            C = cpool.tile([128, 2 * NK], F32)
            nc.vector.tensor_mul(C[:], Craw[:], invf[:])

            # ---------- Horner: s_K = c_K v; s_k = (s_{k+1}+c_k) v ----------
            def horner(col0, tg):
                s = cpool.tile([128, NG], F32, tag=f"h{tg}0")
                nc.vector.tensor_scalar_mul(s[:], v_lay[:],
                                            C[:, col0 + K:col0 + K + 1])
                for k in range(K - 1, 0, -1):
                    t = cpool.tile([128, NG], F32, tag=f"h{tg}{k % 2 + 1}")
                    nc.vector.scalar_tensor_tensor(
                        t[:], s[:], C[:, col0 + k:col0 + k + 1],
                        v_lay[:], op0=ALU.add, op1=ALU.mult)
                    s = t
                r = cpool.tile([128, NG], F32, tag=f"h{tg}3")
                nc.vector.tensor_scalar_add(r[:], s[:], C[:, col0:col0 + 1])
                return r

            den = horner(0, "d")
            num = horner(NK, "n")
            rec = cpool.tile([128, NG], F32)
            nc.vector.reciprocal(rec[:], den[:])
            res = cpool.tile([128, NG], F32)
            nc.vector.tensor_mul(res[:], num[:], rec[:])
            # res[16r+d, g] -> y_out[8g+r, d]: flat = 128g + p
            nc.sync.dma_start(
                bass.AP(out_d[:].tensor, 0, [[1, 128], [128, NG]]), res[:])

    nc.compile()
    return nc


_NC = None


def prep_in_maps(inputs, variant=None):
    variant = variant or VARIANT
    nsl = N // NCORES if variant == "shard" else N
    jw = nsl // 8

    x = np.asarray(inputs["x"], dtype=np.float32)
    calc_X = np.asarray(inputs["calc_X"], dtype=np.float32)
    calc_Y = np.asarray(inputs["calc_Y"], dtype=np.float32)
    W1 = np.asarray(inputs["W1"], dtype=np.float32)
    W2 = np.asarray(inputs["W2"], dtype=np.float32)
    h = float(np.asarray(inputs["h"], dtype=np.float32).reshape(-1)[0])

    XT = np.ascontiguousarray(calc_X.T)                 # [128, 8192]
    W1T = np.ascontiguousarray(W1.T)                    # [128, 256]
    W2Th = np.ascontiguousarray(W2.T) / h               # [256, 16]
    W2Ta = np.ascontiguousarray(W2Th[0:128])
    W2Tb = np.ascontiguousarray(W2Th[128:256])
    sel16 = np.zeros((128, DOUT), dtype=np.float32)
    sel16[np.arange(128), np.arange(128) % DOUT] = 1.0
    fact = np.cumprod(np.concatenate([[1.0], np.arange(1, NK)])).astype(np.float64)
    invf = np.tile((1.0 / fact).astype(np.float32), 2)       # [2*NK]

    in_maps = []
    for c in range(NCORES):
        xTc = np.ascontiguousarray(x[BC * c:BC * (c + 1)].T)   # [128, 64]
        n0 = nsl * c if variant == "shard" else 0
        XTs = np.ascontiguousarray(XT[:, n0:n0 + nsl])
        Ys = calc_Y[n0:n0 + nsl]                               # [nsl, 16]
        Y_lay = np.ascontiguousarray(
            Ys.reshape(8, jw, DOUT).transpose(0, 2, 1).reshape(128, jw))
        in_maps.append({
            "xT": xTc, "XTs": XTs, "Y_lay": Y_lay,
            "W1T": W1T, "W2Ta": W2Ta, "W2Tb": W2Tb,
            "sel16": sel16, "invf": invf,
        })
    return in_maps


def kernel(**inputs):
    global _NC
    in_maps = prep_in_maps(inputs)
    if _NC is None:
        _NC = build_kernel()
    res = run_bass_kernel_spmd(_NC, in_maps, core_ids=list(range(NCORES)))
    out = np.concatenate([res.results[c]["y_out"] for c in range(NCORES)], axis=0)
    return out.astype(np.float32)


if __name__ == "__main__":
    rng = np.random.default_rng(0)
    ins = {
        "x": rng.standard_normal((B, DIN), dtype=np.float32),
        "calc_X": rng.standard_normal((N, DIN), dtype=np.float32),
        "calc_Y": rng.standard_normal((N, DOUT), dtype=np.float32),
        "W1": (rng.standard_normal((DMID, DIN), dtype=np.float32) * DIN ** -0.5),
        "W2": (rng.standard_normal((DOUT, DMID), dtype=np.float32) * DMID ** -0.5),
        "h": np.array([1.5], dtype=np.float32),
    }
    out = kernel(**ins)
    def mlp(v):
        return np.maximum(v @ ins["W1"].T, 0.0) @ ins["W2"].T
    Zw = mlp(ins["x"]); Xw = mlp(ins["calc_X"])
    z = (Xw[None] - Zw[:, None]) / ins["h"][0]
    w = np.exp(-0.5 * z * z)
    ref = (w * ins["calc_Y"][None]).sum(1) / w.sum(1)
    rel = np.abs(out - ref).max() / np.abs(ref).max()
    print("rel err:", rel)
